# revision 1
# baseline (speedup 1.0000x reference)
"""3-layer GAT (DGL-style GATConv) on one TRN2 chip (8 NeuronCores).

Sharding: nodes are range-partitioned across the 8 cores (graph parallel).
Edges are bucketed by destination shard and sorted by destination; each core
owns the edge softmax + message aggregation for its node range.  Per layer,
each core computes its slice of the packed projection [feat | el | er]
(el/er attention dot products folded into the weight matrix on the host),
the slices are AllGather'ed, and per-edge source records are fetched from
the gathered table with indirect DMA.  Scatter-add into destinations is a
matmul with a 0/1 selector matrix built from an iota/is_equal compare.
"""

import os
import sys

import numpy as np

if "/opt/trn_rl_repo" not in sys.path:
    sys.path.insert(0, "/opt/trn_rl_repo")

import ml_dtypes

P = 128            # partitions / block size
NSH = 8            # shards (NeuronCores)
REC = 264          # record width in bf16 units: 256 feat bf16 + 4 el f32
RECF = REC // 2    # record width in f32 units
ELF = 128          # f32-unit offset of el inside a record
NB_MAX = 16        # edge blocks gathered per indirect DMA

# problem constants
N, E = 50000, 800000
IN_DIM, HID, HEADS, OUT_DIM = 512, 256, 4, 256
NEG_SLOPE = 0.2
NS = N // NSH                      # 6250 real nodes per shard
TILES = (NS + P - 1) // P          # 49
NSP = TILES * P                    # 6272 padded nodes per shard


def preprocess_edges(src, dst, ns, nsp, nsh):
    """Bucket edges by dst shard, sort by dst, tile into 128-node dst tiles,
    block into 128-edge blocks.  Block counts per tile are maxed across
    shards so all cores share one instruction stream.

    Returns (nb[t] per tile, B total blocks, and per-core [P, B] arrays:
    src row ids into the gathered table, local dst ids for the er gather,
    and dst-local-in-tile values (-1 for padding) for the selector compare).
    """
    tiles = nsp // P
    src = np.asarray(src).astype(np.int64)
    dst = np.asarray(dst).astype(np.int64)
    shard = dst // ns
    local = dst - shard * ns
    s_sh = src // ns
    prow = (s_sh * nsp + (src - s_sh * ns)).astype(np.int64)  # row in gathered table

    cnt = np.zeros((nsh, tiles), np.int64)
    np.add.at(cnt, (shard, local // P), 1)
    nb = np.maximum(1, -(-cnt // P)).max(axis=0).astype(np.int64)  # ceil, >=1
    B = int(nb.sum())
    base = np.zeros(tiles, np.int64)
    base[1:] = np.cumsum(nb)[:-1]

    src_idx = np.zeros((nsh, P, B), np.int32)
    dstloc = np.full((nsh, P, B), -1.0, np.float32)
    for c in range(nsh):
        m = shard == c
        loc_c = local[m]
        order = np.argsort(loc_c, kind="stable")
        loc_c = loc_c[order]
        prow_c = prow[m][order]
        tile_c = loc_c // P
        for t in range(tiles):
            sel = tile_c == t
            k = int(sel.sum())
            if k == 0:
                continue
            j = np.arange(k)
            pp = j % P
            bb = base[t] + j // P
            src_idx[c, pp, bb] = prow_c[sel]
            dstloc[c, pp, bb] = (loc_c[sel] - t * P).astype(np.float32)
    # dstloc_row: block-major edge-order [B*P]: entry b*P+p = dstloc[p, b]
    dstloc_row = np.ascontiguousarray(dstloc.transpose(0, 2, 1)).reshape(
        nsh, B * P)
    return nb, base, B, src_idx, dstloc, dstloc_row


def pack_weights(W, al, ar):
    """[W | W@blockdiag(al) | W@blockdiag(ar) | zero-pad] -> [k, REC] f32."""
    W = np.asarray(W, np.float32)
    al = np.asarray(al, np.float32)
    ar = np.asarray(ar, np.float32)
    H, D = al.shape
    k = W.shape[0]
    W3 = W.reshape(k, H, D)
    Wel = np.einsum("khd,hd->kh", W3, al)
    Wer = np.einsum("khd,hd->kh", W3, ar)
    pad = np.zeros((k, REC - 256 - 2 * H), np.float32)
    return np.concatenate([W, Wel, Wer, pad], axis=1)


def build_bass(nsp, in_dim, nb, base, B, heads):
    """Build the 3-layer SPMD Bass graph (one graph, 8 cores)."""
    from contextlib import ExitStack

    import concourse.bacc as bacc
    import concourse.bass as bass
    import concourse.mybir as mybir
    import concourse.tile as tile
    from concourse.bass import AP, IndirectOffsetOnAxis
    from concourse.masks import make_identity

    dt = mybir.dt
    f32, bf16, i32 = dt.float32, dt.bfloat16, dt.int32
    AF = mybir.ActivationFunctionType
    tiles = nsp // P
    kdims = [in_dim, 256, 256]

    nc = bacc.Bacc("TRN2", target_bir_lowering=False, debug=False,
                   num_devices=NSH)

    h0T = nc.dram_tensor("h0T", [in_dim, nsp], bf16, kind="ExternalInput")
    wps = [nc.dram_tensor(f"wpack{l}", [kdims[l], REC], bf16,
                          kind="ExternalInput") for l in range(3)]
    bias_d = nc.dram_tensor("biases", [3, 256], f32, kind="ExternalInput")
    src_idx_d = nc.dram_tensor("src_idx", [P, B], i32, kind="ExternalInput")
    dstloc_d = nc.dram_tensor("dstloc", [P, B], bf16, kind="ExternalInput")
    dstrow_d = nc.dram_tensor("dstrow", [1, B * P], bf16,
                              kind="ExternalInput")
    out_d = nc.dram_tensor("out", [nsp, 256], f32, kind="ExternalOutput")

    p_slice = [nc.dram_tensor(f"pslice{l}", [nsp, REC], bf16)
               for l in range(3)]
    p_full = [nc.dram_tensor(f"pfull{l}", [NSH * nsp, REC], bf16,
                             addr_space="Shared") for l in range(3)]

    NBH = int(max(nb))
    with tile.TileContext(nc) as tc, ExitStack() as ctx:
        const = ctx.enter_context(tc.tile_pool(name="const", bufs=1))
        psum_pk = ctx.enter_context(
            tc.tile_pool(name="psum_pk", bufs=2, space="PSUM"))
        psum_ms = ctx.enter_context(
            tc.tile_pool(name="psum_ms", bufs=2, space="PSUM"))
        psum_er = ctx.enter_context(
            tc.tile_pool(name="psum_er", bufs=2, space="PSUM"))
        psum_tr = ctx.enter_context(
            tc.tile_pool(name="psum_tr", bufs=2, space="PSUM"))
        gpool = ctx.enter_context(tc.tile_pool(name="gpool", bufs=2))
        selp = ctx.enter_context(tc.tile_pool(name="selp", bufs=2))
        rpool = ctx.enter_context(tc.tile_pool(name="rpool", bufs=2))
        spool = ctx.enter_context(tc.tile_pool(name="spool", bufs=4))
        mpool = ctx.enter_context(tc.tile_pool(name="mpool", bufs=2))

        # constants / persistent state
        iota_i = const.tile([P, P], i32, name="iota_i", tag="iota_i")
        nc.gpsimd.iota(iota_i[:], pattern=[[1, P]], base=0,
                       channel_multiplier=0)
        iota_bf = const.tile([P, P], bf16, name="iota_bf", tag="iota_bf")
        nc.vector.tensor_copy(iota_bf[:], iota_i[:])
        iotac_i = const.tile([P, 1], i32, name="iotac_i", tag="iotac_i")
        nc.gpsimd.iota(iotac_i[:], pattern=[[1, 1]], base=0,
                       channel_multiplier=1)
        iotac_bf = const.tile([P, 1], bf16, name="iotac_bf", tag="iotac_bf")
        nc.vector.tensor_copy(iotac_bf[:], iotac_i[:])
        ident = const.tile([P, P], bf16, name="ident", tag="ident")
        make_identity(nc, ident[:])

        src_idx_sb = const.tile([P, B], i32, name="srcidx", tag="srcidx")
        nc.sync.dma_start(src_idx_sb[:], src_idx_d[:, :])
        dstloc_sb = const.tile([P, B], bf16, name="dstloc", tag="dstloc")
        nc.sync.dma_start(dstloc_sb[:], dstloc_d[:, :])

        h_in0 = [const.tile([P, nsp], bf16, name=f"h0_{k}", tag=f"h0_{k}")
                 for k in range(in_dim // P)]
        for k in range(in_dim // P):
            nc.sync.dma_start(h_in0[k][:], h0T[k * P:(k + 1) * P, :])
        hT1 = [const.tile([P, nsp], bf16, name=f"h1_{k}", tag=f"h1_{k}")
               for k in range(2)]
        hT2 = [const.tile([P, nsp], bf16, name=f"h2_{k}", tag=f"h2_{k}")
               for k in range(2)]
        h_ins = [h_in0, hT1, hT2]
        h_outs = [hT1, hT2, None]

        for l in range(3):
            H = heads[l]
            DH = 256 // H
            CH = 256 + H           # scatter-matmul rhs cols: [sum | msg]
            kch = kdims[l] // P
            h_in = h_ins[l]
            h_out = h_outs[l]

            w_sb = [const.tile([P, REC], bf16, name=f"w{l}_{k}",
                               tag=f"w{l}_{k}") for k in range(kch)]
            for k in range(kch):
                nc.sync.dma_start(w_sb[k][:], wps[l][k * P:(k + 1) * P, :])
            b_tile = const.tile([P, 256], f32, name=f"btile{l}",
                                tag=f"btile{l}")
            nc.sync.dma_start(b_tile[:],
                              bias_d[l:l + 1, :].to_broadcast((P, 256)))
            er_all = const.tile([P, tiles * H], bf16, name=f"erall{l}",
                                tag=f"erall{l}")

            # ---- pack phase: [feat | el | er] = h @ wpack ----
            for t in range(tiles):
                ps = psum_pk.tile([P, REC], f32, name="pspk", tag="pspk")
                for k in range(kch):
                    nc.tensor.matmul(
                        ps[:], lhsT=h_in[k][:, t * P:(t + 1) * P],
                        rhs=w_sb[k][:], start=(k == 0), stop=(k == kch - 1))
                pack = gpool.tile([P, REC], bf16, name="pack", tag="pack")
                nc.vector.tensor_copy(pack[:, 0:256], ps[:, 0:256])
                pf = pack[:].bitcast(f32)
                el_dst = AP(pf.tensor, pf.offset + ELF, [pf.ap[0], [1, 4]])
                nc.vector.tensor_copy(el_dst, ps[:, 256:260])
                nc.vector.tensor_copy(er_all[:, t * H:(t + 1) * H],
                                      ps[:, 256 + H:256 + 2 * H])
                nc.sync.dma_start(p_slice[l][t * P:(t + 1) * P, :], pack[:])

            # ---- allgather the packed table ----
            nc.gpsimd.collective_compute(
                "AllGather", mybir.AluOpType.bypass,
                replica_groups=[list(range(NSH))],
                ins=[p_slice[l].ap().opt()], outs=[p_full[l].ap().opt()])

            # ---- edge phase ----
            for t in range(tiles):
                nblk = int(nb[t])
                g0 = int(base[t])
                ps_m = psum_ms.tile([P, CH], f32, name="psms", tag="psms")

                # gather source records, one 128-edge block per indirect DMA
                G = gpool.tile([P, NBH * REC], bf16, name="G", tag="G")
                for j in range(nblk):
                    nc.gpsimd.indirect_dma_start(
                        out=G[:, j * REC:(j + 1) * REC], out_offset=None,
                        in_=p_full[l][:, :],
                        in_offset=IndirectOffsetOnAxis(
                            ap=src_idx_sb[:, g0 + j:g0 + j + 1], axis=0))

                # er broadcast: replicate dstloc row, compare vs column iota,
                # then one Nf=H matmul per block against this tile's er rows
                rep = rpool.tile([P, NBH * P], bf16, name="rep", tag="rep")
                nc.sync.dma_start(
                    rep[:, 0:nblk * P],
                    dstrow_d[0:1, g0 * P:(g0 + nblk) * P].to_broadcast(
                        (P, nblk * P)))
                msel = selp.tile([P, NBH * P], bf16, name="msel", tag="msel")
                ioc = iotac_bf[:]
                in1c = AP(ioc.tensor, ioc.offset, [ioc.ap[0], [0, nblk * P]])
                nc.vector.tensor_tensor(out=msel[:, 0:nblk * P],
                                        in0=rep[:, 0:nblk * P], in1=in1c,
                                        op=mybir.AluOpType.is_equal)
                ps_er = psum_er.tile([P, NBH * H], f32, name="pser",
                                     tag="pser")
                for j in range(nblk):
                    nc.tensor.matmul(
                        ps_er[:, j * H:(j + 1) * H],
                        lhsT=msel[:, j * P:(j + 1) * P],
                        rhs=er_all[:, t * H:(t + 1) * H],
                        start=True, stop=True)

                # e = lrelu(el + er); t = exp(e)
                gap = G[:]
                gf = gap.bitcast(f32)
                el_ap = AP(gf.tensor, gf.offset + ELF,
                           [gf.ap[0], [RECF, nblk], [1, H]])
                er3 = ps_er[:, 0:nblk * H].rearrange("p (b h) -> p b h", h=H)
                e1 = spool.tile([P, NBH * H], f32, name="e1", tag="e1")
                e13 = e1[:, 0:nblk * H].rearrange("p (b h) -> p b h", h=H)
                nc.vector.tensor_tensor(out=e13, in0=el_ap, in1=er3,
                                        op=mybir.AluOpType.add)
                e2 = spool.tile([P, NBH * H], f32, name="e2", tag="e2")
                nc.vector.tensor_scalar_mul(
                    e2[:, 0:nblk * H], e1[:, 0:nblk * H], NEG_SLOPE)
                e3 = spool.tile([P, NBH * H], f32, name="e3", tag="e3")
                nc.vector.tensor_tensor(out=e3[:, 0:nblk * H],
                                        in0=e1[:, 0:nblk * H],
                                        in1=e2[:, 0:nblk * H],
                                        op=mybir.AluOpType.max)
                t_bf = spool.tile([P, NBH * H], bf16, name="tbf", tag="tbf")
                nc.scalar.activation(t_bf[:, 0:nblk * H], e3[:, 0:nblk * H],
                                     AF.Exp)

                # scatter selector: dstloc column vs row iota
                sel = selp.tile([P, NBH * P], bf16, name="sel", tag="sel")
                dl = dstloc_sb[:]
                in0 = AP(dl.tensor, dl.offset + g0,
                         [dl.ap[0], [1, nblk], [0, P]])
                io = iota_bf[:]
                in1 = AP(io.tensor, io.offset, [io.ap[0], [0, nblk], [1, P]])
                sel3 = sel[:, 0:nblk * P].rearrange("p (b q) -> p b q", q=P)
                nc.vector.tensor_tensor(out=sel3, in0=in0, in1=in1,
                                        op=mybir.AluOpType.is_equal)

                # rhs = [t | t * feat] per block
                rhs = rpool.tile([P, NBH * CH], bf16, name="rhs", tag="rhs")
                rap = rhs[:]
                t3 = t_bf[:, 0:nblk * H].rearrange("p (b h) -> p b h", h=H)
                s_dst = AP(rap.tensor, rap.offset,
                           [rap.ap[0], [CH, nblk], [1, H]])
                nc.vector.tensor_copy(s_dst, t3)
                gfeat = AP(gap.tensor, gap.offset,
                           [gap.ap[0], [REC, nblk], [DH, H], [1, DH]])
                tb = t_bf[:]
                tmul = AP(tb.tensor, tb.offset,
                          [tb.ap[0], [H, nblk], [1, H], [0, DH]])
                r_dst = AP(rap.tensor, rap.offset + H,
                           [rap.ap[0], [CH, nblk], [DH, H], [1, DH]])
                nc.vector.tensor_tensor(out=r_dst, in0=gfeat, in1=tmul,
                                        op=mybir.AluOpType.mult)

                for j in range(nblk):
                    nc.tensor.matmul(
                        ps_m[:], lhsT=sel[:, j * P:(j + 1) * P],
                        rhs=rhs[:, j * CH:(j + 1) * CH],
                        start=(j == 0), stop=(j == nblk - 1))

                # ---- tile epilogue: msg / sum + bias (+relu, transpose) ----
                s_sb = spool.tile([P, H], f32, name="ssb", tag="ssb")
                nc.vector.tensor_scalar_max(s_sb[:], ps_m[:, 0:H], 1e-30)
                r_sb = spool.tile([P, H], f32, name="rsb", tag="rsb")
                nc.vector.reciprocal(r_sb[:], s_sb[:])
                mn = mpool.tile([P, 256], f32, name="mn", tag="mn")
                mn3 = mn[:].rearrange("p (h d) -> p h d", h=H)
                ms3 = ps_m[:, H:H + 256].rearrange("p (h d) -> p h d", h=H)
                rb = r_sb[:]
                r_bc = AP(rb.tensor, rb.offset, [rb.ap[0], [1, H], [0, DH]])
                nc.vector.tensor_tensor(out=mn3, in0=ms3, in1=r_bc,
                                        op=mybir.AluOpType.mult)
                mb = mpool.tile([P, 256], f32, name="mb", tag="mb")
                nc.vector.tensor_tensor(out=mb[:], in0=mn[:], in1=b_tile[:],
                                        op=mybir.AluOpType.add)
                if l < 2:
                    hb = mpool.tile([P, 256], bf16, name="hb", tag="hb")
                    nc.scalar.activation(hb[:], mb[:], AF.Relu)
                    for k in range(2):
                        pt = psum_tr.tile([P, P], bf16, name="pstr",
                                          tag="pstr")
                        nc.tensor.transpose(pt[:], hb[:, k * P:(k + 1) * P],
                                            ident[:])
                        nc.vector.tensor_copy(
                            h_out[k][:, t * P:(t + 1) * P], pt[:])
                else:
                    nc.sync.dma_start(out_d[t * P:(t + 1) * P, :], mb[:])

    nc.compile()
    return nc


def _make_in_maps(feats, wpacks, biases, nb, base, B,
                  src_idx, dstloc, dstloc_row, ns, nsp, in_dim):
    bf = ml_dtypes.bfloat16
    in_maps = []
    for c in range(NSH):
        sl = np.zeros((nsp, in_dim), np.float32)
        sl[:ns] = feats[c * ns:(c + 1) * ns]
        in_maps.append({
            "h0T": np.ascontiguousarray(sl.T).astype(bf),
            "wpack0": wpacks[0].astype(bf),
            "wpack1": wpacks[1].astype(bf),
            "wpack2": wpacks[2].astype(bf),
            "biases": biases.astype(np.float32),
            "src_idx": np.ascontiguousarray(src_idx[c]),
            "dstloc": dstloc[c].astype(bf),
            "dstrow": dstloc_row[c].reshape(1, -1).astype(bf),
        })
    return in_maps


def gat_host(feats, src, dst, W0, al0, ar0, b0, W1, al1, ar1, b1,
             W2, al2, ar2, b2, ns=NS, nsp=NSP, in_dim=IN_DIM, run=None):
    """Full host flow: preprocess, build, run (via `run` callback), unshard."""
    feats = np.asarray(feats, np.float32)
    heads = [al0.shape[0], al1.shape[0], al2.shape[0]]
    wpacks = [pack_weights(W0, al0, ar0), pack_weights(W1, al1, ar1),
              pack_weights(W2, al2, ar2)]
    biases = np.stack([np.asarray(b0, np.float32),
                       np.asarray(b1, np.float32),
                       np.asarray(b2, np.float32)])
    nb, base, B, src_idx, dstloc, dstloc_row = preprocess_edges(
        src, dst, ns, nsp, NSH)
    nc = build_bass(nsp, in_dim, nb, base, B, heads)
    in_maps = _make_in_maps(feats, wpacks, biases, nb, base, B,
                            src_idx, dstloc, dstloc_row, ns, nsp, in_dim)
    results = run(nc, in_maps)
    out = np.concatenate([results[c]["out"][:ns] for c in range(NSH)], axis=0)
    return np.ascontiguousarray(out.astype(np.float32))


def kernel(**inputs):
    from concourse.bass_utils import run_bass_kernel_spmd

    trace = os.environ.get("GAT_TRACE", "0") == "1"
    tmpdir = os.environ.get("GAT_TRACE_DIR") or None

    def run(nc, in_maps):
        res = run_bass_kernel_spmd(nc, in_maps, core_ids=list(range(NSH)),
                                   trace=trace, tmpdir=tmpdir)
        if trace:
            print(f"HW exec time: {res.exec_time_ns} ns")
        return res.results

    return gat_host(
        inputs["feats"], inputs["src"], inputs["dst"],
        inputs["W0"], inputs["al0"], inputs["ar0"], inputs["b0"],
        inputs["W1"], inputs["al1"], inputs["ar1"], inputs["b1"],
        inputs["W2"], inputs["al2"], inputs["ar2"], inputs["b2"],
        run=run)



# revision 36
# speedup vs baseline: 4.0449x; 4.0449x over previous
"""3-layer GAT (DGL-style GATConv) on one TRN2 chip (8 NeuronCores).

Sharding: nodes are range-partitioned across the 8 cores (graph parallel).
Edges are bucketed by destination shard and sorted by destination; each core
owns the edge softmax + message aggregation for its node range.  Per layer,
each core computes its slice of the packed projection [feat | el | er]
(el/er attention dot products folded into the weight matrix on the host),
the slices are AllGather'ed, and per-edge source records are fetched from
the gathered table with indirect DMA.  Scatter-add into destinations is a
matmul with a 0/1 selector matrix built from an iota/is_equal compare.
"""

import os
import sys

import numpy as np

if "/opt/trn_rl_repo" not in sys.path:
    sys.path.insert(0, "/opt/trn_rl_repo")

import ml_dtypes

P = 128            # partitions / block size
NSH = 8            # shards (NeuronCores)
REC = 264          # record width in bf16 units: 256 feat bf16 + 4 el f32
RECF = REC // 2    # record width in f32 units
ELF = 128          # f32-unit offset of el inside a record
NB_MAX = 16        # edge blocks gathered per indirect DMA

# problem constants
N, E = 50000, 800000
IN_DIM, HID, HEADS, OUT_DIM = 512, 256, 4, 256
NEG_SLOPE = 0.2
NS = N // NSH                      # 6250 real nodes per shard
TILES = (NS + P - 1) // P          # 49
NSP = TILES * P                    # 6272 padded nodes per shard


def preprocess_edges(src, dst, ns, nsp, nsh):
    """Bucket edges by dst shard, sort by dst, tile into 128-node dst tiles,
    block into 128-edge blocks.  Block counts per tile are maxed across
    shards so all cores share one instruction stream.

    Returns (nb[t] per tile, B total blocks, and per-core [P, B] arrays:
    src row ids into the gathered table, local dst ids for the er gather,
    and dst-local-in-tile values (-1 for padding) for the selector compare).
    """
    tiles = nsp // P
    src = np.asarray(src).astype(np.int64)
    dst = np.asarray(dst).astype(np.int64)
    shard = dst // ns
    local = dst - shard * ns
    s_sh = src // ns
    prow = (s_sh * nsp + (src - s_sh * ns)).astype(np.int64)  # row in gathered table

    cnt = np.zeros((nsh, tiles), np.int64)
    np.add.at(cnt, (shard, local // P), 1)
    nb = np.maximum(1, -(-cnt // P)).max(axis=0).astype(np.int64)  # ceil, >=1
    B = int(nb.sum())
    base = np.zeros(tiles, np.int64)
    base[1:] = np.cumsum(nb)[:-1]

    src_idx = np.zeros((nsh, P, B), np.int32)
    dstloc = np.full((nsh, P, B), -1.0, np.float32)
    for c in range(nsh):
        m = shard == c
        loc_c = local[m]
        order = np.argsort(loc_c, kind="stable")
        loc_c = loc_c[order]
        prow_c = prow[m][order]
        tile_c = loc_c // P
        for t in range(tiles):
            sel = tile_c == t
            k = int(sel.sum())
            if k == 0:
                continue
            j = np.arange(k)
            pp = j % P
            bb = base[t] + j // P
            src_idx[c, pp, bb] = prow_c[sel]
            dstloc[c, pp, bb] = (loc_c[sel] - t * P).astype(np.float32)
    # dstloc_row: block-major edge-order [B*P]: entry b*P+p = dstloc[p, b]
    dstloc_row = np.ascontiguousarray(dstloc.transpose(0, 2, 1)).reshape(
        nsh, B * P)
    return nb, base, B, src_idx, dstloc, dstloc_row


def pack_weights(W, al, ar):
    """[W | W@blockdiag(al) | W@blockdiag(ar) | zero-pad] -> [k, REC] f32."""
    W = np.asarray(W, np.float32)
    al = np.asarray(al, np.float32)
    ar = np.asarray(ar, np.float32)
    H, D = al.shape
    k = W.shape[0]
    W3 = W.reshape(k, H, D)
    Wel = np.einsum("khd,hd->kh", W3, al)
    Wer = np.einsum("khd,hd->kh", W3, ar)
    pad = np.zeros((k, REC - 256 - 2 * H), np.float32)
    return np.concatenate([W, Wel, Wer, pad], axis=1)


def build_bass(nsp, in_dim, nb, base, B, heads):
    """Build the 3-layer SPMD Bass graph (one graph, 8 cores)."""
    from contextlib import ExitStack

    import concourse.bacc as bacc
    import concourse.bass as bass
    import concourse.mybir as mybir
    import concourse.tile as tile
    from concourse.bass import AP, IndirectOffsetOnAxis
    from concourse.masks import make_identity

    dt = mybir.dt
    f32, bf16, i32 = dt.float32, dt.bfloat16, dt.int32
    AF = mybir.ActivationFunctionType
    tiles = nsp // P
    kdims = [in_dim, 256, 256]

    nc = bacc.Bacc("TRN2", target_bir_lowering=False, debug=False,
                   num_devices=NSH)

    h0T = nc.dram_tensor("h0T", [in_dim, nsp], bf16, kind="ExternalInput")
    wps = [nc.dram_tensor(f"wpack{l}", [kdims[l], REC], bf16,
                          kind="ExternalInput") for l in range(3)]
    bias_d = nc.dram_tensor("biases", [3, 256], f32, kind="ExternalInput")
    src_idx_d = nc.dram_tensor("src_idx", [P, B], i32, kind="ExternalInput")
    dstloc_d = nc.dram_tensor("dstloc", [P, B], bf16, kind="ExternalInput")
    dstrow_d = nc.dram_tensor("dstrow", [1, B * P], bf16,
                              kind="ExternalInput")
    out_d = nc.dram_tensor("out", [nsp, 256], f32, kind="ExternalOutput")

    p_slice = [nc.dram_tensor(f"pslice{l}", [nsp, REC], bf16)
               for l in range(3)]
    p_full = [nc.dram_tensor(f"pfull{l}", [NSH * nsp, REC], bf16,
                             addr_space="Shared") for l in range(3)]

    NBH = int(max(nb))
    with tile.TileContext(nc) as tc, ExitStack() as ctx:
        const = ctx.enter_context(tc.tile_pool(name="const", bufs=1))
        psum_pk = ctx.enter_context(
            tc.tile_pool(name="psum_pk", bufs=2, space="PSUM"))
        psum_ms = ctx.enter_context(
            tc.tile_pool(name="psum_ms", bufs=2, space="PSUM"))
        psum_er = ctx.enter_context(
            tc.tile_pool(name="psum_er", bufs=2, space="PSUM"))
        psum_tr = ctx.enter_context(
            tc.tile_pool(name="psum_tr", bufs=2, space="PSUM"))
        gpool = ctx.enter_context(tc.tile_pool(name="gpool", bufs=2))
        selp = ctx.enter_context(tc.tile_pool(name="selp", bufs=2))
        rpool = ctx.enter_context(tc.tile_pool(name="rpool", bufs=2))
        spool = ctx.enter_context(tc.tile_pool(name="spool", bufs=4))
        mpool = ctx.enter_context(tc.tile_pool(name="mpool", bufs=2))

        # constants / persistent state
        iota_i = const.tile([P, P], i32, name="iota_i", tag="iota_i")
        nc.gpsimd.iota(iota_i[:], pattern=[[1, P]], base=0,
                       channel_multiplier=0)
        iota_bf = const.tile([P, P], bf16, name="iota_bf", tag="iota_bf")
        nc.vector.tensor_copy(iota_bf[:], iota_i[:])
        iotac_i = const.tile([P, 1], i32, name="iotac_i", tag="iotac_i")
        nc.gpsimd.iota(iotac_i[:], pattern=[[1, 1]], base=0,
                       channel_multiplier=1)
        iotac_bf = const.tile([P, 1], bf16, name="iotac_bf", tag="iotac_bf")
        nc.vector.tensor_copy(iotac_bf[:], iotac_i[:])
        ident = const.tile([P, P], bf16, name="ident", tag="ident")
        make_identity(nc, ident[:])

        src_idx_sb = const.tile([P, B], i32, name="srcidx", tag="srcidx")
        nc.sync.dma_start(src_idx_sb[:], src_idx_d[:, :])
        dstloc_sb = const.tile([P, B], bf16, name="dstloc", tag="dstloc")
        nc.sync.dma_start(dstloc_sb[:], dstloc_d[:, :])

        h_in0 = [const.tile([P, nsp], bf16, name=f"h0_{k}", tag=f"h0_{k}")
                 for k in range(in_dim // P)]
        for k in range(in_dim // P):
            nc.sync.dma_start(h_in0[k][:], h0T[k * P:(k + 1) * P, :])
        hT1 = [const.tile([P, nsp], bf16, name=f"h1_{k}", tag=f"h1_{k}")
               for k in range(2)]
        hT2 = [const.tile([P, nsp], bf16, name=f"h2_{k}", tag=f"h2_{k}")
               for k in range(2)]
        h_ins = [h_in0, hT1, hT2]
        h_outs = [hT1, hT2, None]

        for l in range(3):
            H = heads[l]
            DH = 256 // H
            CH = 256 + H           # scatter-matmul rhs cols: [sum | msg]
            kch = kdims[l] // P
            h_in = h_ins[l]
            h_out = h_outs[l]

            w_sb = [const.tile([P, REC], bf16, name=f"w{l}_{k}",
                               tag=f"w{l}_{k}") for k in range(kch)]
            for k in range(kch):
                nc.sync.dma_start(w_sb[k][:], wps[l][k * P:(k + 1) * P, :])
            b_tile = const.tile([P, 256], f32, name=f"btile{l}",
                                tag=f"btile{l}")
            nc.sync.dma_start(b_tile[:],
                              bias_d[l:l + 1, :].to_broadcast((P, 256)))
            er_all = const.tile([P, tiles * H], bf16, name=f"erall{l}",
                                tag=f"erall{l}")

            # ---- pack phase: [feat | el | er] = h @ wpack ----
            for t in range(tiles):
                ps = psum_pk.tile([P, REC], f32, name="pspk", tag="pspk")
                for k in range(kch):
                    nc.tensor.matmul(
                        ps[:], lhsT=h_in[k][:, t * P:(t + 1) * P],
                        rhs=w_sb[k][:], start=(k == 0), stop=(k == kch - 1))
                pack = gpool.tile([P, REC], bf16, name="pack", tag="pack")
                nc.vector.tensor_copy(pack[:, 0:256], ps[:, 0:256])
                pf = pack[:].bitcast(f32)
                el_dst = AP(pf.tensor, pf.offset + ELF, [pf.ap[0], [1, 4]])
                nc.vector.tensor_copy(el_dst, ps[:, 256:260])
                nc.vector.tensor_copy(er_all[:, t * H:(t + 1) * H],
                                      ps[:, 256 + H:256 + 2 * H])
                nc.sync.dma_start(p_slice[l][t * P:(t + 1) * P, :], pack[:])

            # ---- allgather the packed table ----
            nc.gpsimd.collective_compute(
                "AllGather", mybir.AluOpType.bypass,
                replica_groups=[list(range(NSH))],
                ins=[p_slice[l].ap().opt()], outs=[p_full[l].ap().opt()])

            # ---- edge phase ----
            for t in range(tiles):
                nblk = int(nb[t])
                g0 = int(base[t])
                ps_m = psum_ms.tile([P, CH], f32, name="psms", tag="psms")

                # gather source records, one 128-edge block per indirect DMA
                G = gpool.tile([P, NBH * REC], bf16, name="G", tag="G")
                for j in range(nblk):
                    nc.gpsimd.indirect_dma_start(
                        out=G[:, j * REC:(j + 1) * REC], out_offset=None,
                        in_=p_full[l][:, :],
                        in_offset=IndirectOffsetOnAxis(
                            ap=src_idx_sb[:, g0 + j:g0 + j + 1], axis=0))

                # er broadcast: replicate dstloc row, compare vs column iota,
                # then one Nf=H matmul per block against this tile's er rows
                rep = rpool.tile([P, NBH * P], bf16, name="rep", tag="rep")
                nc.sync.dma_start(
                    rep[:, 0:nblk * P],
                    dstrow_d[0:1, g0 * P:(g0 + nblk) * P].to_broadcast(
                        (P, nblk * P)))
                msel = selp.tile([P, NBH * P], bf16, name="msel", tag="msel")
                ioc = iotac_bf[:]
                in1c = AP(ioc.tensor, ioc.offset, [ioc.ap[0], [0, nblk * P]])
                nc.vector.tensor_tensor(out=msel[:, 0:nblk * P],
                                        in0=rep[:, 0:nblk * P], in1=in1c,
                                        op=mybir.AluOpType.is_equal)
                ps_er = psum_er.tile([P, NBH * H], f32, name="pser",
                                     tag="pser")
                for j in range(nblk):
                    nc.tensor.matmul(
                        ps_er[:, j * H:(j + 1) * H],
                        lhsT=msel[:, j * P:(j + 1) * P],
                        rhs=er_all[:, t * H:(t + 1) * H],
                        start=True, stop=True)

                # e = lrelu(el + er); t = exp(e)
                gap = G[:]
                gf = gap.bitcast(f32)
                el_ap = AP(gf.tensor, gf.offset + ELF,
                           [gf.ap[0], [RECF, nblk], [1, H]])
                er3 = ps_er[:, 0:nblk * H].rearrange("p (b h) -> p b h", h=H)
                e1 = spool.tile([P, NBH * H], f32, name="e1", tag="e1")
                e13 = e1[:, 0:nblk * H].rearrange("p (b h) -> p b h", h=H)
                nc.vector.tensor_tensor(out=e13, in0=el_ap, in1=er3,
                                        op=mybir.AluOpType.add)
                e2 = spool.tile([P, NBH * H], f32, name="e2", tag="e2")
                nc.vector.tensor_scalar_mul(
                    e2[:, 0:nblk * H], e1[:, 0:nblk * H], NEG_SLOPE)
                e3 = spool.tile([P, NBH * H], f32, name="e3", tag="e3")
                nc.vector.tensor_tensor(out=e3[:, 0:nblk * H],
                                        in0=e1[:, 0:nblk * H],
                                        in1=e2[:, 0:nblk * H],
                                        op=mybir.AluOpType.max)
                t_bf = spool.tile([P, NBH * H], bf16, name="tbf", tag="tbf")
                nc.scalar.activation(t_bf[:, 0:nblk * H], e3[:, 0:nblk * H],
                                     AF.Exp)

                # scatter selector: dstloc column vs row iota
                sel = selp.tile([P, NBH * P], bf16, name="sel", tag="sel")
                dl = dstloc_sb[:]
                in0 = AP(dl.tensor, dl.offset + g0,
                         [dl.ap[0], [1, nblk], [0, P]])
                io = iota_bf[:]
                in1 = AP(io.tensor, io.offset, [io.ap[0], [0, nblk], [1, P]])
                sel3 = sel[:, 0:nblk * P].rearrange("p (b q) -> p b q", q=P)
                nc.vector.tensor_tensor(out=sel3, in0=in0, in1=in1,
                                        op=mybir.AluOpType.is_equal)

                # rhs = [t | t * feat] per block
                rhs = rpool.tile([P, NBH * CH], bf16, name="rhs", tag="rhs")
                rap = rhs[:]
                t3 = t_bf[:, 0:nblk * H].rearrange("p (b h) -> p b h", h=H)
                s_dst = AP(rap.tensor, rap.offset,
                           [rap.ap[0], [CH, nblk], [1, H]])
                nc.vector.tensor_copy(s_dst, t3)
                gfeat = AP(gap.tensor, gap.offset,
                           [gap.ap[0], [REC, nblk], [DH, H], [1, DH]])
                tb = t_bf[:]
                tmul = AP(tb.tensor, tb.offset,
                          [tb.ap[0], [H, nblk], [1, H], [0, DH]])
                r_dst = AP(rap.tensor, rap.offset + H,
                           [rap.ap[0], [CH, nblk], [DH, H], [1, DH]])
                nc.vector.tensor_tensor(out=r_dst, in0=gfeat, in1=tmul,
                                        op=mybir.AluOpType.mult)

                for j in range(nblk):
                    nc.tensor.matmul(
                        ps_m[:], lhsT=sel[:, j * P:(j + 1) * P],
                        rhs=rhs[:, j * CH:(j + 1) * CH],
                        start=(j == 0), stop=(j == nblk - 1))

                # ---- tile epilogue: msg / sum + bias (+relu, transpose) ----
                s_sb = spool.tile([P, H], f32, name="ssb", tag="ssb")
                nc.vector.tensor_scalar_max(s_sb[:], ps_m[:, 0:H], 1e-30)
                r_sb = spool.tile([P, H], f32, name="rsb", tag="rsb")
                nc.vector.reciprocal(r_sb[:], s_sb[:])
                mn = mpool.tile([P, 256], f32, name="mn", tag="mn")
                mn3 = mn[:].rearrange("p (h d) -> p h d", h=H)
                ms3 = ps_m[:, H:H + 256].rearrange("p (h d) -> p h d", h=H)
                rb = r_sb[:]
                r_bc = AP(rb.tensor, rb.offset, [rb.ap[0], [1, H], [0, DH]])
                nc.vector.tensor_tensor(out=mn3, in0=ms3, in1=r_bc,
                                        op=mybir.AluOpType.mult)
                mb = mpool.tile([P, 256], f32, name="mb", tag="mb")
                nc.vector.tensor_tensor(out=mb[:], in0=mn[:], in1=b_tile[:],
                                        op=mybir.AluOpType.add)
                if l < 2:
                    hb = mpool.tile([P, 256], bf16, name="hb", tag="hb")
                    nc.scalar.activation(hb[:], mb[:], AF.Relu)
                    for k in range(2):
                        pt = psum_tr.tile([P, P], bf16, name="pstr",
                                          tag="pstr")
                        nc.tensor.transpose(pt[:], hb[:, k * P:(k + 1) * P],
                                            ident[:])
                        nc.vector.tensor_copy(
                            h_out[k][:, t * P:(t + 1) * P], pt[:])
                else:
                    nc.sync.dma_start(out_d[t * P:(t + 1) * P, :], mb[:])

    nc.compile()
    return nc


def _make_in_maps(feats, wpacks, biases, nb, base, B,
                  src_idx, dstloc, dstloc_row, ns, nsp, in_dim):
    bf = ml_dtypes.bfloat16
    in_maps = []
    for c in range(NSH):
        sl = np.zeros((nsp, in_dim), np.float32)
        sl[:ns] = feats[c * ns:(c + 1) * ns]
        in_maps.append({
            "h0T": np.ascontiguousarray(sl.T).astype(bf),
            "wpack0": wpacks[0].astype(bf),
            "wpack1": wpacks[1].astype(bf),
            "wpack2": wpacks[2].astype(bf),
            "biases": biases.astype(np.float32),
            "src_idx": np.ascontiguousarray(src_idx[c]),
            "dstloc": dstloc[c].astype(bf),
            "dstrow": dstloc_row[c].reshape(1, -1).astype(bf),
        })
    return in_maps


def gat_host(feats, src, dst, W0, al0, ar0, b0, W1, al1, ar1, b1,
             W2, al2, ar2, b2, ns=NS, nsp=NSP, in_dim=IN_DIM, run=None):
    """Full host flow: preprocess, build, run (via `run` callback), unshard."""
    feats = np.asarray(feats, np.float32)
    heads = [al0.shape[0], al1.shape[0], al2.shape[0]]
    wpacks = [pack_weights(W0, al0, ar0), pack_weights(W1, al1, ar1),
              pack_weights(W2, al2, ar2)]
    biases = np.stack([np.asarray(b0, np.float32),
                       np.asarray(b1, np.float32),
                       np.asarray(b2, np.float32)])
    nb, base, B, src_idx, dstloc, dstloc_row = preprocess_edges(
        src, dst, ns, nsp, NSH)
    nc = build_bass(nsp, in_dim, nb, base, B, heads)
    in_maps = _make_in_maps(feats, wpacks, biases, nb, base, B,
                            src_idx, dstloc, dstloc_row, ns, nsp, in_dim)
    results = run(nc, in_maps)
    out = np.concatenate([results[c]["out"][:ns] for c in range(NSH)], axis=0)
    return np.ascontiguousarray(out.astype(np.float32))


def kernel(**inputs):
    from concourse.bass_utils import run_bass_kernel_spmd

    trace = os.environ.get("GAT_TRACE", "0") == "1"
    tmpdir = os.environ.get("GAT_TRACE_DIR") or None

    def run(nc, in_maps):
        res = run_bass_kernel_spmd(nc, in_maps, core_ids=list(range(NSH)),
                                   trace=trace, tmpdir=tmpdir)
        if trace:
            print(f"HW exec time: {res.exec_time_ns} ns")
        return res.results

    return gat_host(
        inputs["feats"], inputs["src"], inputs["dst"],
        inputs["W0"], inputs["al0"], inputs["ar0"], inputs["b0"],
        inputs["W1"], inputs["al1"], inputs["ar1"], inputs["b1"],
        inputs["W2"], inputs["al2"], inputs["ar2"], inputs["b2"],
        run=run)
